# revision 7
# baseline (speedup 1.0000x reference)
"""MultiHeadAttention (B=2,N=2048,C=1024,H=16,Dk=64) on 8 TRN2 cores.

Head-tensor-parallel: core c owns heads {2c,2c+1} for both batches.
Device computes qkv^T = Wqkv_s^T @ x^T, causal softmax(q k^T/8) @ v, and the
partial out-projection (rows 128c:128c+128 of W_out); host sums the 8
partials (the "all-reduce"), transposes, and adds the fused bias.
b_k drops (softmax shift invariance); b_v folds into the output bias.

v3: fully interleaved pipeline. Input DMA chunked and overlapped with an
n-outer phase A; attention blocks B(t) emitted right after the A chunk they
need so exps start ~8us in; per-t normalize+out-projection fused in (one
reciprocal_approx_fast per t); scores land as bf16 in single PSUM banks;
diagonal blocks compute only the causally needed query columns; out-proj
pairs two output blocks per PSUM bank / DMA.
"""
import sys

sys.path.insert(0, "/opt/trn_rl_repo")
import numpy as np
import ml_dtypes
import concourse.bass as bass
import concourse.mybir as mybir
from concourse.bass_utils import run_bass_kernel_spmd
from concourse.tile import TileContext

F32 = mybir.dt.float32
F16 = mybir.dt.float16
BF16 = mybir.dt.bfloat16
AF = mybir.ActivationFunctionType
BF = ml_dtypes.bfloat16

T = 4096  # total tokens (2 batches x 2048)
TRACE = False
LAST_EXEC_NS = None
LAST_MEAN_NS = None

_MAX_WAITS = 1  # this neuronxcc build rejects instructions with more sem waits


def _split_excess_waits(nc, limit=_MAX_WAITS):
    """Move excess sem waits onto same-engine nops inserted just before the
    over-subscribed instruction (waits-before-inst on the same queue is
    semantically identical)."""
    ifaces = [nc.tensor, nc.scalar, nc.vector, nc.gpsimd, nc.sync]
    eng_map = {iface.engine: iface for iface in ifaces}
    f = nc.m.functions[0]
    for bb in list(f.blocks):
        il = bb.instructions
        i = 0
        while i < len(il):
            ins = il[i]
            si = ins.sync_info
            waits = list(si.on_wait) if si is not None else []
            if len(waits) > limit:
                keep = waits[-limit:]
                rest = waits[:-limit]
                ins.sync_info = mybir.SyncInfo(
                    on_wait=keep, on_update=list(si.on_update)
                )
                nops = []
                for k in range(0, len(rest), limit):
                    nop = eng_map[ins.engine].nop(nofuse=True)
                    nop.ins.sync_info = mybir.SyncInfo(
                        on_wait=rest[k : k + limit], on_update=[]
                    )
                    nops.append(nop.ins)
                for ni in nops:
                    for bb2 in list(f.blocks):
                        try:
                            bb2.instructions.remove(ni)
                            break
                        except ValueError:
                            pass
                for off, ni in enumerate(nops):
                    il.insert(i + off, ni)
                i += len(nops)
            i += 1


def _build():
    nc = bass.Bass("TRN2", target_bir_lowering=False, debug=False, num_devices=8)
    xt_d = nc.declare_dram_parameter("xt", (1024, T), BF16, isOutput=False)
    wqkv_d = nc.declare_dram_parameter("wqkv", (1024, 384), BF16, isOutput=False)
    bq_d = nc.declare_dram_parameter("bq", (128, 1), F32, isOutput=False)
    wout_d = nc.declare_dram_parameter("wout", (128, 1024), BF16, isOutput=False)
    tri_d = nc.declare_dram_parameter("tri", (128, 128), BF16, isOutput=False)
    s21_d = nc.declare_dram_parameter("s21", (1, 128), BF16, isOutput=False)
    s22_d = nc.declare_dram_parameter("s22", (1, 128), BF16, isOutput=False)
    ident_d = nc.declare_dram_parameter("ident", (128, 128), BF16, isOutput=False)
    outp_d = nc.declare_dram_parameter("outp", (1024, T), F16, isOutput=True)

    with TileContext(nc) as tc:
        with tc.tile_pool(name="sb", bufs=1) as sb, tc.tile_pool(
            name="ps", bufs=1, space="PSUM"
        ) as ps:
            # ---- persistent tiles ----
            wq_t = [
                sb.tile((128, 384), BF16, tag=f"wq{kc}", name=f"wq{kc}")
                for kc in range(8)
            ]
            # per-(kc, n-pair) chunk tiles so deps stay chunk-granular
            xt_t = [
                [
                    sb.tile(
                        (128, 1024), BF16, tag=f"xt{kc}_{np}", name=f"xt{kc}_{np}"
                    )
                    for np in range(4)
                ]
                for kc in range(8)
            ]
            bq_t = sb.tile((128, 1), F32, tag="bq")
            wout_t = sb.tile((128, 1024), BF16, tag="wout")
            tri_t = sb.tile((128, 128), BF16, tag="tri")
            id_t = sb.tile((128, 128), BF16, tag="ident")
            s21_t = sb.tile((1, 128), BF16, tag="s21")
            s22_t = sb.tile((1, 128), BF16, tag="s22")

            # ---- input DMAs: small tensors + weights on otherwise-idle
            # engine queues, xt chunks on sync/gpsimd in consumption order ----
            nc.scalar.dma_start(bq_t[:], bq_d[:, :])
            nc.scalar.dma_start(tri_t[:], tri_d[:, :])
            nc.scalar.dma_start(id_t[:], ident_d[:, :])
            nc.scalar.dma_start(s21_t[:], s21_d[:, :])
            nc.scalar.dma_start(s22_t[:], s22_d[:, :])
            nc.scalar.dma_start(wout_t[:], wout_d[:, :])
            for kc in range(8):
                nc.scalar.dma_start(wq_t[kc][:], wqkv_d[128 * kc : 128 * kc + 128, :])
            qi = 0
            for np_ in range(4):
                for kc in range(8):
                    eng = nc.sync if qi % 2 == 0 else nc.gpsimd
                    qi += 1
                    eng.dma_start(
                        xt_t[kc][np_][:],
                        xt_d[
                            128 * kc : 128 * kc + 128,
                            1024 * np_ : 1024 * np_ + 1024,
                        ],
                    )

            q_T = sb.tile((128, T), BF16, tag="q_T")
            k_T = sb.tile((128, T), BF16, tag="k_T")
            v_T = sb.tile((128, T), BF16, tag="v_T")
            vext = [
                sb.tile((128, 2080), BF16, tag=f"vext{b}", name=f"vext{b}")
                for b in range(2)
            ]
            nc.gpsimd.memset(vext[0][:], 1.0)
            nc.gpsimd.memset(vext[1][:], 1.0)

            # diagonal es buffers: only cols >= 128r are ever written/read
            es_diag = [
                sb.tile((128, 1024), BF16, tag=f"esd{r}", name=f"esd{r}")
                for r in range(4)
            ]

            dst = [q_T, k_T, v_T]

            def emit_A(n):
                """qkv^T for token chunk n (512 tokens) + v transposes."""
                np_, half = divmod(n, 2)
                for m in range(3):
                    a = ps.tile((128, 512), F32, tag="a", bufs=1)
                    for kc in range(8):
                        nc.tensor.matmul(
                            a[:],
                            wq_t[kc][:, 128 * m : 128 * m + 128],
                            xt_t[kc][np_][:, 512 * half : 512 * half + 512],
                            start=(kc == 0),
                            stop=(kc == 7),
                        )
                    o = dst[m][:, 512 * n : 512 * n + 512]
                    if m == 0:
                        nc.vector.tensor_scalar_add(o, a[:], bq_t[:, 0:1])
                    else:
                        nc.vector.tensor_copy(o, a[:])
                for w in range(4):
                    tglob = 4 * n + w
                    b, jj = divmod(tglob, 16)
                    trp = ps.tile((128, 128), BF16, tag="aux", bufs=1)
                    nc.tensor.transpose(
                        trp[:], v_T[:, 128 * tglob : 128 * tglob + 128], id_t[:]
                    )
                    # single DVE copy lands both halves: out chunks at 65*jj
                    # and 65*(16+jj) (stride 1040)
                    c0 = 65 * jj
                    oslc = vext[b][:, c0 : c0 + 64]
                    islc = trp[:]
                    o_ap = bass.AP(
                        oslc.tensor,
                        oslc.offset,
                        [[oslc.ap[0][0], oslc.ap[0][1]], [1040, 2], [1, 64]],
                    )
                    i_ap = bass.AP(
                        islc.tensor,
                        islc.offset,
                        [[islc.ap[0][0], islc.ap[0][1]], [64, 2], [1, 64]],
                    )
                    nc.vector.tensor_copy(o_ap, i_ap)

            def emit_B(t):
                """Causal attention + normalize + partial out-proj for query
                block t (512 queries)."""
                b, i = divmod(t, 4)
                nj = 4 * i + 4
                qs = 2048 * b + 512 * i
                av = ps.tile((65, 1024), F32, tag="av", bufs=1)
                for jj in range(nj):
                    r = jj - 4 * i
                    ks = 2048 * b + 128 * jj
                    sps = ps.tile((128, 1024), F32, tag="sps", bufs=2)
                    if r < 0:
                        for hl in range(2):
                            nc.tensor.matmul(
                                sps[:, 512 * hl : 512 * hl + 512],
                                k_T[64 * hl : 64 * hl + 64, ks : ks + 128],
                                q_T[64 * hl : 64 * hl + 64, qs : qs + 512],
                                start=True,
                                stop=True,
                                skip_group_check=True,
                            )
                        es = sb.tile((128, 1024), BF16, tag="es", bufs=3)
                        nc.scalar.activation(es[:], sps[:], AF.Exp, scale=0.125)
                        for hl in range(2):
                            c = 65 * (16 * hl + jj)
                            nc.tensor.matmul(
                                av[:, 512 * hl : 512 * hl + 512],
                                vext[b][:, c : c + 65],
                                es[:, 512 * hl : 512 * hl + 512],
                                start=(jj == 0),
                                stop=(jj == nj - 1),
                                skip_group_check=True,
                            )
                    else:
                        # diagonal: only query cols >= 128r are causally live
                        w = 512 - 128 * r
                        for hl in range(2):
                            nc.tensor.matmul(
                                sps[:, 512 * hl + 128 * r : 512 * hl + 512],
                                k_T[64 * hl : 64 * hl + 64, ks : ks + 128],
                                q_T[
                                    64 * hl : 64 * hl + 64,
                                    qs + 128 * r : qs + 512,
                                ],
                                start=True,
                                stop=True,
                                skip_group_check=True,
                            )
                        es = es_diag[r]
                        if r == 0:
                            nc.scalar.activation(
                                es[:], sps[:], AF.Exp, scale=0.125
                            )
                        else:
                            # one 2-chunk ACT call covers both hl halves
                            oslc = es[:, 128 * r : 128 * r + w]
                            islc = sps[:, 128 * r : 128 * r + w]
                            o_ap = bass.AP(
                                oslc.tensor,
                                oslc.offset,
                                [
                                    [oslc.ap[0][0], oslc.ap[0][1]],
                                    [512, 2],
                                    [1, w],
                                ],
                            )
                            i_ap = bass.AP(
                                islc.tensor,
                                islc.offset,
                                [
                                    [islc.ap[0][0], islc.ap[0][1]],
                                    [512, 2],
                                    [1, w],
                                ],
                            )
                            nc.scalar.activation(o_ap, i_ap, AF.Exp, scale=0.125)
                        for hl in range(2):
                            c0 = 512 * hl + 128 * r
                            nc.vector.tensor_mul(
                                es[:, c0 : c0 + 128],
                                es[:, c0 : c0 + 128],
                                tri_t[:],
                            )
                        for hl in range(2):
                            c = 65 * (16 * hl + jj)
                            nc.tensor.matmul(
                                av[
                                    :,
                                    512 * hl + 128 * r : 512 * hl + 512,
                                ],
                                vext[b][:, c : c + 65],
                                es[:, 512 * hl + 128 * r : 512 * hl + 512],
                                start=(jj == 0),
                                stop=(jj == nj - 1),
                                skip_group_check=True,
                            )
                # per-t normalize: one fast reciprocal of the denominator row
                # straight from PSUM (64->0 partition shift, quadrant-aligned),
                # broadcast via two K=1 matmuls
                rec = sb.tile((1, 1024), F32, tag="rec", bufs=2)
                nc.vector.reciprocal(rec[:], av[64:65, :])
                rb = sb.tile((1, 1024), BF16, tag="rb", bufs=2)
                nc.vector.tensor_copy(rb[:], rec[:])
                bcp = ps.tile((128, 512), F32, tag="aux", bufs=1)
                nc.tensor.matmul(
                    bcp[:],
                    s21_t[:],
                    rb[:, 0:512],
                    start=True,
                    stop=False,
                    skip_group_check=True,
                )
                nc.tensor.matmul(
                    bcp[:],
                    s22_t[:],
                    rb[:, 512:1024],
                    start=False,
                    stop=True,
                    skip_group_check=True,
                )
                bcps = sb.tile((128, 512), BF16, tag="bcps", bufs=2)
                nc.vector.tensor_copy(bcps[:], bcp[:])
                attnT = sb.tile((128, 512), BF16, tag="attnT", bufs=2)
                for hl in range(2):
                    nc.vector.tensor_mul(
                        attnT[64 * hl : 64 * hl + 64, :],
                        av[0:64, 512 * hl : 512 * hl + 512],
                        bcps[64 * hl : 64 * hl + 64, :],
                    )
                for mp in range(4):
                    osb = sb.tile((128, 1024), F16, tag="osb", bufs=3)
                    for half in range(2):
                        mo = 2 * mp + half
                        op = ps.tile((128, 512), F32, tag="aux", bufs=1)
                        nc.tensor.matmul(
                            op[:],
                            wout_t[:, 128 * mo : 128 * mo + 128],
                            attnT[:],
                            start=True,
                            stop=True,
                            skip_group_check=True,
                        )
                        oh = osb[:, 512 * half : 512 * half + 512]
                        if mp == 3 or (mp == 1 and half == 1):
                            nc.scalar.activation(oh, op[:], AF.Copy)
                        else:
                            nc.vector.tensor_copy(oh, op[:])
                    # batched store: SBUF halves -> two 128-row DRAM blocks
                    dslc = outp_d[256 * mp : 256 * mp + 128, qs : qs + 512]
                    d_ap = bass.AP(
                        dslc.tensor,
                        dslc.offset,
                        [[dslc.ap[0][0], dslc.ap[0][1]], [128 * T, 2], [1, 512]],
                    )
                    s_ap = bass.AP(
                        osb.tensor,
                        osb.offset,
                        [[osb.ap[0][0], osb.ap[0][1]], [512, 2], [1, 512]],
                    )
                    eng = nc.sync if mp % 2 == 0 else nc.gpsimd
                    eng.dma_start(d_ap, s_ap)

            # interleaved emission: each B(t) right after the A chunk it needs
            for n in range(8):
                emit_A(n)
                emit_B(n)
    _split_excess_waits(nc)
    return nc


def kernel(**inputs):
    global LAST_EXEC_NS, LAST_MEAN_NS
    x = np.asarray(inputs["x"], np.float32)
    Wqkv = np.asarray(inputs["W_qkv"], np.float32)
    bqkv = np.asarray(inputs["b_qkv"], np.float32)
    Wout = np.asarray(inputs["W_out"], np.float32)
    bout = np.asarray(inputs["b_out"], np.float32)

    xt = np.ascontiguousarray(x.reshape(T, 1024).T).astype(BF)
    kk = np.arange(128)[:, None]
    qq = np.arange(128)[None, :]
    tri = (qq >= kk).astype(BF)
    ident = np.eye(128).astype(BF)
    s21 = np.zeros((1, 128), BF)
    s21[0, 0:64] = 1.0
    s22 = np.zeros((1, 128), BF)
    s22[0, 64:128] = 1.0

    in_maps = []
    for c in range(8):
        s = 128 * c
        wq = np.ascontiguousarray(
            np.concatenate(
                [
                    Wqkv[:, s : s + 128],
                    Wqkv[:, 1024 + s : 1024 + s + 128],
                    Wqkv[:, 2048 + s : 2048 + s + 128],
                ],
                axis=1,
            )
        ).astype(BF)
        in_maps.append(
            {
                "xt": xt,
                "wqkv": wq,
                "bq": np.ascontiguousarray(
                    bqkv[s : s + 128].reshape(128, 1)
                ).astype(np.float32),
                "wout": np.ascontiguousarray(Wout[s : s + 128, :]).astype(BF),
                "tri": tri,
                "s21": s21,
                "s22": s22,
                "ident": ident,
            }
        )

    nc = _build()
    res = run_bass_kernel_spmd(nc, in_maps, list(range(8)), trace=TRACE)
    LAST_EXEC_NS = res.exec_time_ns
    LAST_MEAN_NS = res.mean_exec_time_ns

    total = np.zeros((1024, T), np.float32)
    for c in range(8):
        total += np.asarray(res.results[c]["outp"]).astype(np.float32)
    beff = (
        bout.astype(np.float64) + bqkv[2048:].astype(np.float64) @ Wout.astype(np.float64)
    ).astype(np.float32)
    out = total.T.reshape(2, 2048, 1024) + beff
    return out.astype(np.float32)


# revision 10
# speedup vs baseline: 1.0534x; 1.0534x over previous
"""MultiHeadAttention (B=2,N=2048,C=1024,H=16,Dk=64) on 8 TRN2 cores.

Head-tensor-parallel: core c owns heads {2c,2c+1} for both batches.
Device computes qkv^T = Wqkv_s^T @ x^T, causal softmax(q k^T/8) @ v, and the
partial out-projection (rows 128c:128c+128 of W_out); host sums the 8
partials (the "all-reduce"), transposes, and adds the fused bias.
b_k drops (softmax shift invariance); b_v folds into the output bias.

v3: fully interleaved pipeline. Input DMA chunked and overlapped with an
n-outer phase A; attention blocks B(t) emitted right after the A chunk they
need so exps start ~8us in; per-t normalize+out-projection fused in (one
reciprocal_approx_fast per t); scores land as bf16 in single PSUM banks;
diagonal blocks compute only the causally needed query columns; out-proj
pairs two output blocks per PSUM bank / DMA.
"""
import sys

sys.path.insert(0, "/opt/trn_rl_repo")
import numpy as np
import ml_dtypes
import concourse.bass as bass
import concourse.mybir as mybir
from concourse.bass_utils import run_bass_kernel_spmd
from concourse.tile import TileContext

F32 = mybir.dt.float32
F16 = mybir.dt.float16
BF16 = mybir.dt.bfloat16
AF = mybir.ActivationFunctionType
BF = ml_dtypes.bfloat16

T = 4096  # total tokens (2 batches x 2048)
TRACE = False
LAST_EXEC_NS = None
LAST_MEAN_NS = None

_MAX_WAITS = 1  # this neuronxcc build rejects instructions with more sem waits


def _split_excess_waits(nc, limit=_MAX_WAITS):
    """Move excess sem waits onto same-engine nops inserted just before the
    over-subscribed instruction (waits-before-inst on the same queue is
    semantically identical)."""
    ifaces = [nc.tensor, nc.scalar, nc.vector, nc.gpsimd, nc.sync]
    eng_map = {iface.engine: iface for iface in ifaces}
    f = nc.m.functions[0]
    for bb in list(f.blocks):
        il = bb.instructions
        i = 0
        while i < len(il):
            ins = il[i]
            si = ins.sync_info
            waits = list(si.on_wait) if si is not None else []
            if len(waits) > limit:
                keep = waits[-limit:]
                rest = waits[:-limit]
                ins.sync_info = mybir.SyncInfo(
                    on_wait=keep, on_update=list(si.on_update)
                )
                nops = []
                for k in range(0, len(rest), limit):
                    nop = eng_map[ins.engine].nop(nofuse=True)
                    nop.ins.sync_info = mybir.SyncInfo(
                        on_wait=rest[k : k + limit], on_update=[]
                    )
                    nops.append(nop.ins)
                for ni in nops:
                    for bb2 in list(f.blocks):
                        try:
                            bb2.instructions.remove(ni)
                            break
                        except ValueError:
                            pass
                for off, ni in enumerate(nops):
                    il.insert(i + off, ni)
                i += len(nops)
            i += 1


def _build():
    nc = bass.Bass("TRN2", target_bir_lowering=False, debug=False, num_devices=8)
    xt_d = nc.declare_dram_parameter("xt", (1024, T), BF16, isOutput=False)
    wqkv_d = nc.declare_dram_parameter("wqkv", (1024, 384), BF16, isOutput=False)
    bq_d = nc.declare_dram_parameter("bq", (128, 1), F32, isOutput=False)
    wout_d = nc.declare_dram_parameter("wout", (128, 1024), BF16, isOutput=False)
    tri_d = nc.declare_dram_parameter("tri", (128, 128), BF16, isOutput=False)
    s21_d = nc.declare_dram_parameter("s21", (1, 128), BF16, isOutput=False)
    s22_d = nc.declare_dram_parameter("s22", (1, 128), BF16, isOutput=False)
    ident_d = nc.declare_dram_parameter("ident", (128, 128), BF16, isOutput=False)
    outp_d = nc.declare_dram_parameter("outp", (1024, T), F16, isOutput=True)

    with TileContext(nc) as tc:
        with tc.tile_pool(name="sb", bufs=1) as sb, tc.tile_pool(
            name="ps", bufs=1, space="PSUM"
        ) as ps:
            # ---- persistent tiles ----
            wq_t = [
                sb.tile((128, 384), BF16, tag=f"wq{kc}", name=f"wq{kc}")
                for kc in range(8)
            ]
            # per-(kc, n-pair) chunk tiles so deps stay chunk-granular
            xt_t = [
                [
                    sb.tile(
                        (128, 1024), BF16, tag=f"xt{kc}_{np}", name=f"xt{kc}_{np}"
                    )
                    for np in range(4)
                ]
                for kc in range(8)
            ]
            bq_t = sb.tile((128, 1), F32, tag="bq")
            wout_t = sb.tile((128, 1024), BF16, tag="wout")
            tri_t = sb.tile((128, 128), BF16, tag="tri")
            id_t = sb.tile((128, 128), BF16, tag="ident")
            s21_t = sb.tile((1, 128), BF16, tag="s21")
            s22_t = sb.tile((1, 128), BF16, tag="s22")

            # ---- input DMAs: small tensors + weights on otherwise-idle
            # engine queues, xt chunks on sync/gpsimd in consumption order ----
            nc.scalar.dma_start(bq_t[:], bq_d[:, :])
            nc.scalar.dma_start(tri_t[:], tri_d[:, :])
            nc.scalar.dma_start(id_t[:], ident_d[:, :])
            nc.scalar.dma_start(s21_t[:], s21_d[:, :])
            nc.scalar.dma_start(s22_t[:], s22_d[:, :])
            nc.scalar.dma_start(wout_t[:], wout_d[:, :])
            for kc in range(8):
                nc.scalar.dma_start(wq_t[kc][:], wqkv_d[128 * kc : 128 * kc + 128, :])
            qi = 0
            for np_ in range(4):
                for kc in range(8):
                    eng = nc.sync if qi % 2 == 0 else nc.gpsimd
                    qi += 1
                    eng.dma_start(
                        xt_t[kc][np_][:],
                        xt_d[
                            128 * kc : 128 * kc + 128,
                            1024 * np_ : 1024 * np_ + 1024,
                        ],
                    )

            q_T = sb.tile((128, T), BF16, tag="q_T")
            k_T = sb.tile((128, T), BF16, tag="k_T")
            v_T = sb.tile((128, T), BF16, tag="v_T")
            vext = [
                sb.tile((128, 2080), BF16, tag=f"vext{b}", name=f"vext{b}")
                for b in range(2)
            ]
            nc.gpsimd.memset(vext[0][:], 1.0)
            nc.gpsimd.memset(vext[1][:], 1.0)
            warm = sb.tile((1, 16), F32, tag="warm")
            nc.vector.memset(warm[:], 1.0)
            nc.scalar.activation(warm[:], warm[:], AF.Ln)
            nc.scalar.activation(warm[:], warm[:], AF.Exp)

            # diagonal es buffers: only cols >= 128r are ever written/read
            es_diag = [
                sb.tile((128, 1024), BF16, tag=f"esd{r}", name=f"esd{r}")
                for r in range(4)
            ]

            dst = [q_T, k_T, v_T]

            def emit_A(n):
                """qkv^T for token chunk n (512 tokens) + v transposes."""
                np_, half = divmod(n, 2)
                for m in range(3):
                    a = ps.tile((128, 512), F32, tag="a", bufs=1)
                    for kc in range(8):
                        nc.tensor.matmul(
                            a[:],
                            wq_t[kc][:, 128 * m : 128 * m + 128],
                            xt_t[kc][np_][:, 512 * half : 512 * half + 512],
                            start=(kc == 0),
                            stop=(kc == 7),
                        )
                    o = dst[m][:, 512 * n : 512 * n + 512]
                    if m == 0:
                        nc.vector.tensor_scalar_add(o, a[:], bq_t[:, 0:1])
                    else:
                        nc.vector.tensor_copy(o, a[:])
                for w in range(4):
                    tglob = 4 * n + w
                    b, jj = divmod(tglob, 16)
                    trp = ps.tile((128, 128), BF16, tag="aux", bufs=1)
                    nc.tensor.transpose(
                        trp[:], v_T[:, 128 * tglob : 128 * tglob + 128], id_t[:]
                    )
                    # single DVE copy lands both halves: out chunks at 65*jj
                    # and 65*(16+jj) (stride 1040)
                    c0 = 65 * jj
                    oslc = vext[b][:, c0 : c0 + 64]
                    islc = trp[:]
                    o_ap = bass.AP(
                        oslc.tensor,
                        oslc.offset,
                        [[oslc.ap[0][0], oslc.ap[0][1]], [1040, 2], [1, 64]],
                    )
                    i_ap = bass.AP(
                        islc.tensor,
                        islc.offset,
                        [[islc.ap[0][0], islc.ap[0][1]], [64, 2], [1, 64]],
                    )
                    nc.vector.tensor_copy(o_ap, i_ap)

            def emit_B(t):
                """Causal attention + normalize + partial out-proj for query
                block t (512 queries)."""
                b, i = divmod(t, 4)
                nj = 4 * i + 4
                qs = 2048 * b + 512 * i
                av = ps.tile((65, 1024), F32, tag="av", bufs=1)
                for jj in range(nj):
                    r = jj - 4 * i
                    ks = 2048 * b + 128 * jj
                    sps = ps.tile((128, 1024), F32, tag="sps", bufs=2)
                    if r < 0:
                        for hl in range(2):
                            nc.tensor.matmul(
                                sps[:, 512 * hl : 512 * hl + 512],
                                k_T[64 * hl : 64 * hl + 64, ks : ks + 128],
                                q_T[64 * hl : 64 * hl + 64, qs : qs + 512],
                                start=True,
                                stop=True,
                                skip_group_check=True,
                            )
                        es = sb.tile((128, 1024), BF16, tag="es", bufs=3)
                        nc.scalar.activation(es[:], sps[:], AF.Exp, scale=0.125)
                        for hl in range(2):
                            c = 65 * (16 * hl + jj)
                            nc.tensor.matmul(
                                av[:, 512 * hl : 512 * hl + 512],
                                vext[b][:, c : c + 65],
                                es[:, 512 * hl : 512 * hl + 512],
                                start=(jj == 0),
                                stop=(jj == nj - 1),
                                skip_group_check=True,
                            )
                    else:
                        # diagonal: only query cols >= 128r are causally live
                        w = 512 - 128 * r
                        for hl in range(2):
                            nc.tensor.matmul(
                                sps[:, 512 * hl + 128 * r : 512 * hl + 512],
                                k_T[64 * hl : 64 * hl + 64, ks : ks + 128],
                                q_T[
                                    64 * hl : 64 * hl + 64,
                                    qs + 128 * r : qs + 512,
                                ],
                                start=True,
                                stop=True,
                                skip_group_check=True,
                            )
                        es = es_diag[r]
                        if r == 0:
                            nc.scalar.activation(
                                es[:], sps[:], AF.Exp, scale=0.125
                            )
                        else:
                            # one 2-chunk ACT call covers both hl halves
                            oslc = es[:, 128 * r : 128 * r + w]
                            islc = sps[:, 128 * r : 128 * r + w]
                            o_ap = bass.AP(
                                oslc.tensor,
                                oslc.offset,
                                [
                                    [oslc.ap[0][0], oslc.ap[0][1]],
                                    [512, 2],
                                    [1, w],
                                ],
                            )
                            i_ap = bass.AP(
                                islc.tensor,
                                islc.offset,
                                [
                                    [islc.ap[0][0], islc.ap[0][1]],
                                    [512, 2],
                                    [1, w],
                                ],
                            )
                            nc.scalar.activation(o_ap, i_ap, AF.Exp, scale=0.125)
                        for hl in range(2):
                            c0 = 512 * hl + 128 * r
                            nc.vector.tensor_mul(
                                es[:, c0 : c0 + 128],
                                es[:, c0 : c0 + 128],
                                tri_t[:],
                            )
                        for hl in range(2):
                            c = 65 * (16 * hl + jj)
                            nc.tensor.matmul(
                                av[
                                    :,
                                    512 * hl + 128 * r : 512 * hl + 512,
                                ],
                                vext[b][:, c : c + 65],
                                es[:, 512 * hl + 128 * r : 512 * hl + 512],
                                start=(jj == 0),
                                stop=(jj == nj - 1),
                                skip_group_check=True,
                            )
                # per-t normalize: evacuate av head-stacked (frees the
                # PSUM bank fast), then 1/den via ACT ln -> Dekker bf16
                # broadcast matmuls -> ACT exp(-x) which lands bcps directly
                av_sb = sb.tile((128, 512), F32, tag="avsb", bufs=2)
                for hl in range(2):
                    nc.vector.tensor_copy(
                        av_sb[64 * hl : 64 * hl + 64, :],
                        av[0:64, 512 * hl : 512 * hl + 512],
                    )
                den = sb.tile((1, 1024), F32, tag="den", bufs=2)
                nc.vector.tensor_copy(den[:], av[64:65, :])
                lden = sb.tile((1, 1024), F32, tag="lden", bufs=2)
                nc.scalar.activation(lden[:], den[:], AF.Ln)
                ldh = sb.tile((1, 1024), BF16, tag="ldh", bufs=2)
                nc.vector.tensor_copy(ldh[:], lden[:])
                ldhf = sb.tile((1, 1024), F32, tag="ldhf", bufs=2)
                nc.vector.tensor_copy(ldhf[:], ldh[:])
                ldl = sb.tile((1, 1024), BF16, tag="ldl", bufs=2)
                nc.vector.tensor_sub(ldl[:], lden[:], ldhf[:])
                bcp = ps.tile((128, 512), F32, tag="aux", bufs=1)
                nc.tensor.matmul(
                    bcp[:], s21_t[:], ldh[:, 0:512],
                    start=True, stop=False, skip_group_check=True,
                )
                nc.tensor.matmul(
                    bcp[:], s21_t[:], ldl[:, 0:512],
                    start=False, stop=False, skip_group_check=True,
                )
                nc.tensor.matmul(
                    bcp[:], s22_t[:], ldh[:, 512:1024],
                    start=False, stop=False, skip_group_check=True,
                )
                nc.tensor.matmul(
                    bcp[:], s22_t[:], ldl[:, 512:1024],
                    start=False, stop=True, skip_group_check=True,
                )
                bcps = sb.tile((128, 512), BF16, tag="bcps", bufs=2)
                nc.scalar.activation(bcps[:], bcp[:], AF.Exp, scale=-1.0)
                attnT = sb.tile((128, 512), BF16, tag="attnT", bufs=2)
                for hl in range(2):
                    nc.vector.tensor_mul(
                        attnT[64 * hl : 64 * hl + 64, :],
                        av_sb[64 * hl : 64 * hl + 64, :],
                        bcps[64 * hl : 64 * hl + 64, :],
                    )
                for mp in range(4):
                    osb = sb.tile((128, 1024), F16, tag="osb", bufs=3)
                    for half in range(2):
                        mo = 2 * mp + half
                        op = ps.tile((128, 512), F32, tag="aux", bufs=1)
                        nc.tensor.matmul(
                            op[:],
                            wout_t[:, 128 * mo : 128 * mo + 128],
                            attnT[:],
                            start=True,
                            stop=True,
                            skip_group_check=True,
                        )
                        oh = osb[:, 512 * half : 512 * half + 512]
                        nc.vector.tensor_copy(oh, op[:])
                    # batched store: SBUF halves -> two 128-row DRAM blocks
                    dslc = outp_d[256 * mp : 256 * mp + 128, qs : qs + 512]
                    d_ap = bass.AP(
                        dslc.tensor,
                        dslc.offset,
                        [[dslc.ap[0][0], dslc.ap[0][1]], [128 * T, 2], [1, 512]],
                    )
                    s_ap = bass.AP(
                        osb.tensor,
                        osb.offset,
                        [[osb.ap[0][0], osb.ap[0][1]], [512, 2], [1, 512]],
                    )
                    eng = nc.sync if mp % 2 == 0 else nc.gpsimd
                    eng.dma_start(d_ap, s_ap)

            # interleaved emission: each B(t) right after the A chunk it needs
            for n in range(8):
                emit_A(n)
                emit_B(n)
    _split_excess_waits(nc)
    return nc


def kernel(**inputs):
    global LAST_EXEC_NS, LAST_MEAN_NS
    x = np.asarray(inputs["x"], np.float32)
    Wqkv = np.asarray(inputs["W_qkv"], np.float32)
    bqkv = np.asarray(inputs["b_qkv"], np.float32)
    Wout = np.asarray(inputs["W_out"], np.float32)
    bout = np.asarray(inputs["b_out"], np.float32)

    xt = np.ascontiguousarray(x.reshape(T, 1024).T).astype(BF)
    kk = np.arange(128)[:, None]
    qq = np.arange(128)[None, :]
    tri = (qq >= kk).astype(BF)
    ident = np.eye(128).astype(BF)
    s21 = np.zeros((1, 128), BF)
    s21[0, 0:64] = 1.0
    s22 = np.zeros((1, 128), BF)
    s22[0, 64:128] = 1.0

    in_maps = []
    for c in range(8):
        s = 128 * c
        wq = np.ascontiguousarray(
            np.concatenate(
                [
                    Wqkv[:, s : s + 128],
                    Wqkv[:, 1024 + s : 1024 + s + 128],
                    Wqkv[:, 2048 + s : 2048 + s + 128],
                ],
                axis=1,
            )
        ).astype(BF)
        in_maps.append(
            {
                "xt": xt,
                "wqkv": wq,
                "bq": np.ascontiguousarray(
                    bqkv[s : s + 128].reshape(128, 1)
                ).astype(np.float32),
                "wout": np.ascontiguousarray(Wout[s : s + 128, :]).astype(BF),
                "tri": tri,
                "s21": s21,
                "s22": s22,
                "ident": ident,
            }
        )

    nc = _build()
    res = run_bass_kernel_spmd(nc, in_maps, list(range(8)), trace=TRACE)
    LAST_EXEC_NS = res.exec_time_ns
    LAST_MEAN_NS = res.mean_exec_time_ns

    total = np.zeros((1024, T), np.float32)
    for c in range(8):
        total += np.asarray(res.results[c]["outp"]).astype(np.float32)
    beff = (
        bout.astype(np.float64) + bqkv[2048:].astype(np.float64) @ Wout.astype(np.float64)
    ).astype(np.float32)
    out = total.T.reshape(2, 2048, 1024) + beff
    return out.astype(np.float32)


# revision 12
# speedup vs baseline: 1.0835x; 1.0285x over previous
"""MultiHeadAttention (B=2,N=2048,C=1024,H=16,Dk=64) on 8 TRN2 cores.

Head-tensor-parallel: core c owns heads {2c,2c+1} for both batches.
Device computes qkv^T = Wqkv_s^T @ x^T, causal softmax(q k^T/8) @ v, and the
partial out-projection (rows 128c:128c+128 of W_out); host sums the 8
partials (the "all-reduce"), transposes, and adds the fused bias.
b_k drops (softmax shift invariance); b_v folds into the output bias.

v3: fully interleaved pipeline. Input DMA chunked and overlapped with an
n-outer phase A; attention blocks B(t) emitted right after the A chunk they
need so exps start ~8us in; per-t normalize+out-projection fused in (one
reciprocal_approx_fast per t); scores land as bf16 in single PSUM banks;
diagonal blocks compute only the causally needed query columns; out-proj
pairs two output blocks per PSUM bank / DMA.
"""
import sys

sys.path.insert(0, "/opt/trn_rl_repo")
import numpy as np
import ml_dtypes
import concourse.bass as bass
import concourse.mybir as mybir
from concourse.bass_utils import run_bass_kernel_spmd
from concourse.tile import TileContext

F32 = mybir.dt.float32
F16 = mybir.dt.float16
BF16 = mybir.dt.bfloat16
AF = mybir.ActivationFunctionType
BF = ml_dtypes.bfloat16

T = 4096  # total tokens (2 batches x 2048)
TRACE = False
LAST_EXEC_NS = None
LAST_MEAN_NS = None

_MAX_WAITS = 1  # this neuronxcc build rejects instructions with more sem waits


def _split_excess_waits(nc, limit=_MAX_WAITS):
    """Move excess sem waits onto same-engine nops inserted just before the
    over-subscribed instruction (waits-before-inst on the same queue is
    semantically identical)."""
    ifaces = [nc.tensor, nc.scalar, nc.vector, nc.gpsimd, nc.sync]
    eng_map = {iface.engine: iface for iface in ifaces}
    f = nc.m.functions[0]
    for bb in list(f.blocks):
        il = bb.instructions
        i = 0
        while i < len(il):
            ins = il[i]
            si = ins.sync_info
            waits = list(si.on_wait) if si is not None else []
            if len(waits) > limit:
                keep = waits[-limit:]
                rest = waits[:-limit]
                ins.sync_info = mybir.SyncInfo(
                    on_wait=keep, on_update=list(si.on_update)
                )
                nops = []
                for k in range(0, len(rest), limit):
                    nop = eng_map[ins.engine].nop(nofuse=True)
                    nop.ins.sync_info = mybir.SyncInfo(
                        on_wait=rest[k : k + limit], on_update=[]
                    )
                    nops.append(nop.ins)
                for ni in nops:
                    for bb2 in list(f.blocks):
                        try:
                            bb2.instructions.remove(ni)
                            break
                        except ValueError:
                            pass
                for off, ni in enumerate(nops):
                    il.insert(i + off, ni)
                i += len(nops)
            i += 1


def _build():
    nc = bass.Bass("TRN2", target_bir_lowering=False, debug=False, num_devices=8)
    xt_d = nc.declare_dram_parameter("xt", (1024, T), BF16, isOutput=False)
    wqkv_d = nc.declare_dram_parameter("wqkv", (1024, 384), BF16, isOutput=False)
    bq_d = nc.declare_dram_parameter("bq", (128, 1), F32, isOutput=False)
    wout_d = nc.declare_dram_parameter("wout", (128, 1024), BF16, isOutput=False)
    tri_d = nc.declare_dram_parameter("tri", (128, 128), BF16, isOutput=False)
    s21_d = nc.declare_dram_parameter("s21", (1, 128), BF16, isOutput=False)
    s22_d = nc.declare_dram_parameter("s22", (1, 128), BF16, isOutput=False)
    ident_d = nc.declare_dram_parameter("ident", (128, 128), BF16, isOutput=False)
    outp_d = nc.declare_dram_parameter("outp", (1024, T), F16, isOutput=True)

    with TileContext(nc) as tc:
        with tc.tile_pool(name="sb", bufs=1) as sb, tc.tile_pool(
            name="ps", bufs=1, space="PSUM"
        ) as ps:
            # ---- persistent tiles ----
            wq_t = [
                sb.tile((128, 384), BF16, tag=f"wq{kc}", name=f"wq{kc}")
                for kc in range(8)
            ]
            # per-(kc, n-pair) chunk tiles so deps stay chunk-granular
            xt_t = [
                [
                    sb.tile(
                        (128, 1024), BF16, tag=f"xt{kc}_{np}", name=f"xt{kc}_{np}"
                    )
                    for np in range(4)
                ]
                for kc in range(8)
            ]
            bq_t = sb.tile((128, 1), F32, tag="bq")
            wout_t = sb.tile((128, 1024), BF16, tag="wout")
            tri_t = sb.tile((128, 128), BF16, tag="tri")
            id_t = sb.tile((128, 128), BF16, tag="ident")
            s21_t = sb.tile((1, 128), BF16, tag="s21")
            s22_t = sb.tile((1, 128), BF16, tag="s22")

            # ---- input DMAs: small tensors + weights on otherwise-idle
            # engine queues, xt chunks on sync/gpsimd in consumption order ----
            nc.scalar.dma_start(bq_t[:], bq_d[:, :])
            nc.scalar.dma_start(tri_t[:], tri_d[:, :])
            nc.scalar.dma_start(id_t[:], ident_d[:, :])
            nc.scalar.dma_start(s21_t[:], s21_d[:, :])
            nc.scalar.dma_start(s22_t[:], s22_d[:, :])
            nc.scalar.dma_start(wout_t[:], wout_d[:, :])
            for kc in range(8):
                nc.scalar.dma_start(wq_t[kc][:], wqkv_d[128 * kc : 128 * kc + 128, :])
            qi = 0
            for kc in range(8):
                eng = nc.sync if qi % 2 == 0 else nc.gpsimd
                qi += 1
                eng.dma_start(
                    xt_t[kc][0][:, 0:512],
                    xt_d[128 * kc : 128 * kc + 128, 0:512],
                )
            for kc in range(8):
                eng = nc.sync if qi % 2 == 0 else nc.gpsimd
                qi += 1
                eng.dma_start(
                    xt_t[kc][0][:, 512:1024],
                    xt_d[128 * kc : 128 * kc + 128, 512:1024],
                )
            for np_ in range(1, 4):
                for kc in range(8):
                    eng = nc.sync if qi % 2 == 0 else nc.gpsimd
                    qi += 1
                    eng.dma_start(
                        xt_t[kc][np_][:],
                        xt_d[
                            128 * kc : 128 * kc + 128,
                            1024 * np_ : 1024 * np_ + 1024,
                        ],
                    )

            q_T = sb.tile((128, T), BF16, tag="q_T")
            k_T = sb.tile((128, T), BF16, tag="k_T")
            v_T = sb.tile((128, T), BF16, tag="v_T")
            vext = [
                sb.tile((128, 2080), BF16, tag=f"vext{b}", name=f"vext{b}")
                for b in range(2)
            ]
            nc.gpsimd.memset(vext[0][:], 1.0)
            nc.gpsimd.memset(vext[1][:], 1.0)
            warm = sb.tile((1, 16), F32, tag="warm")
            nc.vector.memset(warm[:], 1.0)
            nc.scalar.activation(warm[:], warm[:], AF.Ln)
            nc.scalar.activation(warm[:], warm[:], AF.Exp)

            # diagonal es buffers: only cols >= 128r are ever written/read
            es_diag = [
                sb.tile((128, 1024), BF16, tag=f"esd{r}", name=f"esd{r}")
                for r in range(4)
            ]

            dst = [q_T, k_T, v_T]

            def emit_A(n):
                """qkv^T for token chunk n (512 tokens) + v transposes."""
                np_, half = divmod(n, 2)
                for m in range(3):
                    a = ps.tile((128, 512), F32, tag="a", bufs=1)
                    for kc in range(8):
                        nc.tensor.matmul(
                            a[:],
                            wq_t[kc][:, 128 * m : 128 * m + 128],
                            xt_t[kc][np_][:, 512 * half : 512 * half + 512],
                            start=(kc == 0),
                            stop=(kc == 7),
                        )
                    o = dst[m][:, 512 * n : 512 * n + 512]
                    if m == 0:
                        nc.vector.tensor_scalar_add(o, a[:], bq_t[:, 0:1])
                    else:
                        nc.vector.tensor_copy(o, a[:])
                for w in range(4):
                    tglob = 4 * n + w
                    b, jj = divmod(tglob, 16)
                    trp = ps.tile((128, 128), BF16, tag="aux", bufs=1)
                    nc.tensor.transpose(
                        trp[:], v_T[:, 128 * tglob : 128 * tglob + 128], id_t[:]
                    )
                    # single DVE copy lands both halves: out chunks at 65*jj
                    # and 65*(16+jj) (stride 1040)
                    c0 = 65 * jj
                    oslc = vext[b][:, c0 : c0 + 64]
                    islc = trp[:]
                    o_ap = bass.AP(
                        oslc.tensor,
                        oslc.offset,
                        [[oslc.ap[0][0], oslc.ap[0][1]], [1040, 2], [1, 64]],
                    )
                    i_ap = bass.AP(
                        islc.tensor,
                        islc.offset,
                        [[islc.ap[0][0], islc.ap[0][1]], [64, 2], [1, 64]],
                    )
                    nc.vector.tensor_copy(o_ap, i_ap)

            def emit_B(t):
                """Causal attention + normalize + partial out-proj for query
                block t (512 queries)."""
                b, i = divmod(t, 4)
                nj = 4 * i + 4
                qs = 2048 * b + 512 * i
                av = ps.tile((65, 1024), F32, tag="av", bufs=1)
                for jj in range(nj):
                    r = jj - 4 * i
                    ks = 2048 * b + 128 * jj
                    sps = ps.tile((128, 1024), F32, tag="sps", bufs=2)
                    if r < 0:
                        for hl in range(2):
                            nc.tensor.matmul(
                                sps[:, 512 * hl : 512 * hl + 512],
                                k_T[64 * hl : 64 * hl + 64, ks : ks + 128],
                                q_T[64 * hl : 64 * hl + 64, qs : qs + 512],
                                start=True,
                                stop=True,
                                skip_group_check=True,
                            )
                        es = sb.tile((128, 1024), BF16, tag="es", bufs=3)
                        nc.scalar.activation(es[:], sps[:], AF.Exp, scale=0.125)
                        for hl in range(2):
                            c = 65 * (16 * hl + jj)
                            nc.tensor.matmul(
                                av[:, 512 * hl : 512 * hl + 512],
                                vext[b][:, c : c + 65],
                                es[:, 512 * hl : 512 * hl + 512],
                                start=(jj == 0),
                                stop=(jj == nj - 1),
                                skip_group_check=True,
                            )
                    else:
                        # diagonal: only query cols >= 128r are causally live
                        w = 512 - 128 * r
                        for hl in range(2):
                            nc.tensor.matmul(
                                sps[:, 512 * hl + 128 * r : 512 * hl + 512],
                                k_T[64 * hl : 64 * hl + 64, ks : ks + 128],
                                q_T[
                                    64 * hl : 64 * hl + 64,
                                    qs + 128 * r : qs + 512,
                                ],
                                start=True,
                                stop=True,
                                skip_group_check=True,
                            )
                        es = es_diag[r]
                        if r == 0:
                            nc.scalar.activation(
                                es[:], sps[:], AF.Exp, scale=0.125
                            )
                        else:
                            # one 2-chunk ACT call covers both hl halves
                            oslc = es[:, 128 * r : 128 * r + w]
                            islc = sps[:, 128 * r : 128 * r + w]
                            o_ap = bass.AP(
                                oslc.tensor,
                                oslc.offset,
                                [
                                    [oslc.ap[0][0], oslc.ap[0][1]],
                                    [512, 2],
                                    [1, w],
                                ],
                            )
                            i_ap = bass.AP(
                                islc.tensor,
                                islc.offset,
                                [
                                    [islc.ap[0][0], islc.ap[0][1]],
                                    [512, 2],
                                    [1, w],
                                ],
                            )
                            nc.scalar.activation(o_ap, i_ap, AF.Exp, scale=0.125)
                        for hl in range(2):
                            c0 = 512 * hl + 128 * r
                            nc.vector.tensor_mul(
                                es[:, c0 : c0 + 128],
                                es[:, c0 : c0 + 128],
                                tri_t[:],
                            )
                        for hl in range(2):
                            c = 65 * (16 * hl + jj)
                            nc.tensor.matmul(
                                av[
                                    :,
                                    512 * hl + 128 * r : 512 * hl + 512,
                                ],
                                vext[b][:, c : c + 65],
                                es[:, 512 * hl + 128 * r : 512 * hl + 512],
                                start=(jj == 0),
                                stop=(jj == nj - 1),
                                skip_group_check=True,
                            )
                # per-t normalize: evacuate av head-stacked (frees the
                # PSUM bank fast), then 1/den via ACT ln -> Dekker bf16
                # broadcast matmuls -> ACT exp(-x) which lands bcps directly
                av_sb = sb.tile((128, 512), F32, tag="avsb", bufs=2)
                for hl in range(2):
                    nc.vector.tensor_copy(
                        av_sb[64 * hl : 64 * hl + 64, :],
                        av[0:64, 512 * hl : 512 * hl + 512],
                    )
                den = sb.tile((1, 1024), F32, tag="den", bufs=2)
                nc.vector.tensor_copy(den[:], av[64:65, :])
                lden = sb.tile((1, 1024), F32, tag="lden", bufs=2)
                nc.scalar.activation(lden[:], den[:], AF.Ln)
                ldh = sb.tile((1, 1024), BF16, tag="ldh", bufs=2)
                nc.vector.tensor_copy(ldh[:], lden[:])
                ldhf = sb.tile((1, 1024), F32, tag="ldhf", bufs=2)
                nc.vector.tensor_copy(ldhf[:], ldh[:])
                ldl = sb.tile((1, 1024), BF16, tag="ldl", bufs=2)
                nc.vector.tensor_sub(ldl[:], lden[:], ldhf[:])
                bcp = ps.tile((128, 512), F32, tag="aux", bufs=1)
                nc.tensor.matmul(
                    bcp[:], s21_t[:], ldh[:, 0:512],
                    start=True, stop=False, skip_group_check=True,
                )
                nc.tensor.matmul(
                    bcp[:], s21_t[:], ldl[:, 0:512],
                    start=False, stop=False, skip_group_check=True,
                )
                nc.tensor.matmul(
                    bcp[:], s22_t[:], ldh[:, 512:1024],
                    start=False, stop=False, skip_group_check=True,
                )
                nc.tensor.matmul(
                    bcp[:], s22_t[:], ldl[:, 512:1024],
                    start=False, stop=True, skip_group_check=True,
                )
                bcps = sb.tile((128, 512), BF16, tag="bcps", bufs=2)
                nc.scalar.activation(bcps[:], bcp[:], AF.Exp, scale=-1.0)
                attnT = sb.tile((128, 512), BF16, tag="attnT", bufs=2)
                for hl in range(2):
                    nc.vector.tensor_mul(
                        attnT[64 * hl : 64 * hl + 64, :],
                        av_sb[64 * hl : 64 * hl + 64, :],
                        bcps[64 * hl : 64 * hl + 64, :],
                    )
                for mp in range(4):
                    osb = sb.tile((128, 1024), F16, tag="osb", bufs=3)
                    for half in range(2):
                        mo = 2 * mp + half
                        optag = ("a" if mo % 2 == 1 else "aux") if t == 7 else "aux"
                        op = ps.tile((128, 512), F32, tag=optag, bufs=1)
                        nc.tensor.matmul(
                            op[:],
                            wout_t[:, 128 * mo : 128 * mo + 128],
                            attnT[:],
                            start=True,
                            stop=True,
                            skip_group_check=True,
                        )
                        oh = osb[:, 512 * half : 512 * half + 512]
                        nc.vector.tensor_copy(oh, op[:])
                    # batched store: SBUF halves -> two 128-row DRAM blocks
                    dslc = outp_d[256 * mp : 256 * mp + 128, qs : qs + 512]
                    d_ap = bass.AP(
                        dslc.tensor,
                        dslc.offset,
                        [[dslc.ap[0][0], dslc.ap[0][1]], [128 * T, 2], [1, 512]],
                    )
                    s_ap = bass.AP(
                        osb.tensor,
                        osb.offset,
                        [[osb.ap[0][0], osb.ap[0][1]], [512, 2], [1, 512]],
                    )
                    eng = nc.sync if mp % 2 == 0 else nc.gpsimd
                    eng.dma_start(d_ap, s_ap)

            # interleaved emission: each B(t) right after the A chunk it needs
            for n in range(8):
                emit_A(n)
                emit_B(n)
    _split_excess_waits(nc)
    return nc


def kernel(**inputs):
    global LAST_EXEC_NS, LAST_MEAN_NS
    x = np.asarray(inputs["x"], np.float32)
    Wqkv = np.asarray(inputs["W_qkv"], np.float32)
    bqkv = np.asarray(inputs["b_qkv"], np.float32)
    Wout = np.asarray(inputs["W_out"], np.float32)
    bout = np.asarray(inputs["b_out"], np.float32)

    xt = np.ascontiguousarray(x.reshape(T, 1024).T).astype(BF)
    kk = np.arange(128)[:, None]
    qq = np.arange(128)[None, :]
    tri = (qq >= kk).astype(BF)
    ident = np.eye(128).astype(BF)
    s21 = np.zeros((1, 128), BF)
    s21[0, 0:64] = 1.0
    s22 = np.zeros((1, 128), BF)
    s22[0, 64:128] = 1.0

    in_maps = []
    for c in range(8):
        s = 128 * c
        wq = np.ascontiguousarray(
            np.concatenate(
                [
                    Wqkv[:, s : s + 128],
                    Wqkv[:, 1024 + s : 1024 + s + 128],
                    Wqkv[:, 2048 + s : 2048 + s + 128],
                ],
                axis=1,
            )
        ).astype(BF)
        in_maps.append(
            {
                "xt": xt,
                "wqkv": wq,
                "bq": np.ascontiguousarray(
                    bqkv[s : s + 128].reshape(128, 1)
                ).astype(np.float32),
                "wout": np.ascontiguousarray(Wout[s : s + 128, :]).astype(BF),
                "tri": tri,
                "s21": s21,
                "s22": s22,
                "ident": ident,
            }
        )

    nc = _build()
    res = run_bass_kernel_spmd(nc, in_maps, list(range(8)), trace=TRACE)
    LAST_EXEC_NS = res.exec_time_ns
    LAST_MEAN_NS = res.mean_exec_time_ns

    total = np.zeros((1024, T), np.float32)
    for c in range(8):
        total += np.asarray(res.results[c]["outp"]).astype(np.float32)
    beff = (
        bout.astype(np.float64) + bqkv[2048:].astype(np.float64) @ Wout.astype(np.float64)
    ).astype(np.float32)
    out = total.T.reshape(2, 2048, 1024) + beff
    return out.astype(np.float32)
